# revision 22
# baseline (speedup 1.0000x reference)
"""Causal self-attention (B=4, T=2048, D=1024, H=16) on 8 trn2 NeuronCores.

Sharding: batch (4-way) x head-half (2-way tensor parallel) => 8 cores,
one uniform SPMD program (per-core differences are pure data: which batch's
x, which half of the QKV columns / proj columns each core receives).

Per core (batch b, head-half hh, 8 local heads), all matmul operands bf16
(fp32 PSUM accumulation; the fp32 alternative on trn2 PE is the fp32-
emulation path at 3-4 cycles/row, and f32r is its truncated high pass --
same effective operand precision as bf16 but 3x slower):
  1. QKV: q^T/k^T computed in [qkv_col, token] layout (lhsT = W chunk,
     rhs = x^T chunk); v computed in [token, vcol] layout.
  2. Attention per head, per 512-wide query tile, streaming 128-wide key
     blocks (block-causal; fully-masked key blocks are skipped):
       S^T[k,q]  = matmul(lhsT=k^T chunk, rhs=q^T tile)      (PSUM f32)
       P^T       = exp(S^T / 8) on ScalarE (PSUM -> SBUF bf16)
       diagonal blocks: causal zeroing via gpsimd affine_select
       y_ext^T  += matmul(lhsT=v_ext block, rhs=P^T): v_ext carries a ones
                   column, so row HD of the accumulator is the softmax
                   denominator l -- no extra reduction pass.
       normalization: l rows of all 8 heads batched into one DVE
       reciprocal, broadcast across partitions via a K=1 ones matmul,
       one DVE multiply per head.
  3. Pairwise AllGather of y^T (bf16, 512x512 per query tile) between the
     two cores sharing a batch => full y^T [1024, 512] on both.
  4. proj: out[:, 512 cols of this core] = y @ W_p[:, cols] (+bias),
     column-sharded => the host only concatenates, no reduction anywhere.
"""

import os
import sys
from dataclasses import dataclass

import ml_dtypes
import numpy as np

sys.path.insert(0, "/opt/trn_rl_repo")

import concourse.mybir as mybir  # noqa: E402
import concourse.tile as tile  # noqa: E402
from concourse import bacc  # noqa: E402
from concourse.bass import ds, ts  # noqa: E402

P = 128
F32 = mybir.dt.float32
BF16 = mybir.dt.bfloat16
AF = mybir.ActivationFunctionType
ALU = mybir.AluOpType
BF16NP = ml_dtypes.bfloat16


@dataclass(frozen=True)
class Cfg:
    T: int = 2048          # sequence length
    D: int = 1024          # model dim (QKV contraction dim)
    H_LOC: int = 8         # heads per core
    HD: int = 64           # head dim
    TT: int = 512          # token tile width in the QKV phase
    QT: int = 512          # query tile width in the attention phase
    n_groups: int = 2      # cores sharing a batch (pairwise AllGather)
    scale: float = 64 ** -0.5

    @property
    def DH(self):          # local head dims (y^T rows contributed per core)
        return self.H_LOC * self.HD

    @property
    def GDH(self):         # proj contraction dim (= model dim)
        return self.n_groups * self.DH

    @property
    def DCH(self):
        return self.D // P

    @property
    def NHP(self):         # 128-partition groups of local head dims
        return self.DH // P

    @property
    def HPG(self):         # heads per 128-partition group
        return P // self.HD

    @property
    def NTT(self):
        return self.T // self.TT

    @property
    def NQT(self):
        return self.T // self.QT

    @property
    def CB(self):          # 128-wide column blocks of the local q/k cols
        return self.DH // P


FULL = Cfg()


def build_nc(c: Cfg, n_cores: int = 8):
    """Build the (uniform SPMD) Bass program for one core."""
    assert c.T % c.TT == 0 and c.T % c.QT == 0 and c.QT % P == 0
    assert c.D % P == 0 and c.DH % P == 0 and c.TT % P == 0
    use_cc = c.n_groups > 1

    nc = bacc.Bacc(
        "TRN2", target_bir_lowering=False, debug=False, num_devices=n_cores
    )
    xT = nc.dram_tensor("xT", [c.D, c.T], BF16, kind="ExternalInput").ap()
    wq = nc.dram_tensor("wq", [c.D, c.DH], BF16, kind="ExternalInput").ap()
    wk = nc.dram_tensor("wk", [c.D, c.DH], BF16, kind="ExternalInput").ap()
    wv = nc.dram_tensor("wv", [c.D, c.DH], BF16, kind="ExternalInput").ap()
    bq = nc.dram_tensor("bq", [c.DH], F32, kind="ExternalInput").ap()
    bk = nc.dram_tensor("bk", [c.DH], F32, kind="ExternalInput").ap()
    bv = nc.dram_tensor("bv", [1, c.DH], BF16, kind="ExternalInput").ap()
    wp = nc.dram_tensor("wp", [c.GDH, c.DH], BF16, kind="ExternalInput").ap()
    bp = nc.dram_tensor("bp", [1, c.DH], BF16, kind="ExternalInput").ap()
    oc = max(P, (c.T // P) * c.H_LOC)
    onesin = nc.dram_tensor("onesin", [P, oc], BF16, kind="ExternalInput").ap()
    esel = nc.dram_tensor("esel", [c.H_LOC, c.NHP * P], BF16,
                          kind="ExternalInput").ap()
    out = nc.dram_tensor("out", [c.T, c.DH], F32, kind="ExternalOutput").ap()

    groups = [[g * c.n_groups + i for i in range(c.n_groups)]
              for g in range(max(1, n_cores // c.n_groups))]

    with tile.TileContext(nc) as tc:
        with (
            tc.tile_pool(name="const", bufs=1) as cst,
            tc.tile_pool(name="kv", bufs=1) as kv,
            tc.tile_pool(name="wproj", bufs=1) as wpp,
            tc.tile_pool(name="ps_mm", bufs=2, space="PSUM") as ps_mm,
            tc.tile_pool(name="ps_s", bufs=4, space="PSUM") as ps_s,
            tc.tile_pool(name="ps_y", bufs=2, space="PSUM") as ps_y,
            tc.tile_pool(name="dram", bufs=2, space="DRAM") as drp,
        ):
            # ---- constants ----
            ones_row = cst.tile([1, P], BF16)
            nc.gpsimd.dma_start(ones_row, onesin[0:1, 0:P])
            bq_sb = cst.tile([P, c.CB], F32)
            nc.scalar.dma_start(bq_sb, bq.rearrange("(cb p) -> p cb", p=P))
            bk_sb = cst.tile([P, c.CB], F32)
            nc.scalar.dma_start(bk_sb, bk.rearrange("(cb p) -> p cb", p=P))
            bv_row = cst.tile([1, c.DH], BF16)
            nc.scalar.dma_start(bv_row, bv)
            bp_row = cst.tile([1, c.DH], BF16)
            nc.scalar.dma_start(bp_row, bp)
            esel_sb = cst.tile([c.H_LOC, c.NHP * P], BF16)
            nc.gpsimd.dma_start(esel_sb, esel)
            # causal masks for the diagonal-region key blocks: mask[o][k,j]
            # keeps where j - k >= o*128 (j = query within tile, k = key)
            ndiag = c.QT // P
            mask4 = cst.tile([P, ndiag, c.QT], BF16)
            nc.vector.memset(mask4, 1.0)
            for o in range(ndiag):
                nc.gpsimd.affine_select(
                    mask4[:, o, :], mask4[:, o, :],
                    compare_op=ALU.is_ge, fill=0.0, base=-o * P,
                    pattern=[[1, c.QT]], channel_multiplier=-1,
                )

            # ---- persistent K^T / Q^T / V(+ones) ----
            kT = kv.tile([P, c.NHP, c.T], BF16)
            qT = kv.tile([P, c.NHP, c.T], BF16)
            v = kv.tile([P, c.T // P, c.H_LOC, c.HD + 1], BF16)
            nc.vector.memset(v[:, :, :, c.HD:c.HD + 1], 1.0)

            wp_sb = wpp.tile([P, c.GDH // P, c.DH], BF16)
            nc.gpsimd.dma_start(
                wp_sb, wp.rearrange("(ch p) n -> p ch n", p=P))

            # ================= QKV phase =================
            with (
                tc.tile_pool(name="wqkv", bufs=1) as wqk,
                tc.tile_pool(name="xt", bufs=2) as xtp,
            ):
                wq_sb = wqk.tile([P, c.DCH, c.DH], BF16)
                wk_sb = wqk.tile([P, c.DCH, c.DH], BF16)
                wv_sb = wqk.tile([P, c.DCH, c.DH], BF16)
                wr = {"wq": wq.rearrange("(ch p) n -> p ch n", p=P),
                      "wk": wk.rearrange("(ch p) n -> p ch n", p=P),
                      "wv": wv.rearrange("(ch p) n -> p ch n", p=P)}
                for dc in range(c.DCH):
                    nc.gpsimd.dma_start(wk_sb[:, dc, :], wr["wk"][:, dc, :])
                    nc.scalar.dma_start(wq_sb[:, dc, :], wr["wq"][:, dc, :])
                    nc.gpsimd.dma_start(wv_sb[:, dc, :], wr["wv"][:, dc, :])

                xT_r = xT.rearrange("(ch p) t -> p ch t", p=P)
                for tt in range(c.NTT):
                    xt = xtp.tile([P, c.DCH, c.TT], BF16)
                    nc.sync.dma_start(xt, xT_r[:, :, ts(tt, c.TT)])

                    # K^T and Q^T: [col, token] layout
                    for dst, w_sb, b_sb in (
                        (kT, wk_sb, bk_sb),
                        (qT, wq_sb, bq_sb),
                    ):
                        for cb in range(c.CB):
                            pst = ps_mm.tile([P, max(c.TT, c.DH)], F32,
                                             tag="mm", name="pst")[:, :c.TT]
                            for dc in range(c.DCH):
                                nc.tensor.matmul(
                                    pst,
                                    w_sb[:, dc, ts(cb, P)],
                                    xt[:, dc, :],
                                    start=(dc == 0),
                                    stop=(dc == c.DCH - 1),
                                )
                            nc.vector.tensor_tensor(
                                dst[:, cb, ts(tt, c.TT)], pst,
                                b_sb[:, cb:cb + 1].to_broadcast(
                                    (P, c.TT)),
                                ALU.add,
                            )

                    # V: [token, vcol] layout (+ bias via ones-row matmul)
                    for tb in range(c.TT // P):
                        gtb = tt * (c.TT // P) + tb
                        psv = ps_mm.tile([P, max(c.TT, c.DH)], F32,
                                         tag="mm", name="psv")[:, :c.DH]
                        for dc in range(c.DCH):
                            nc.tensor.matmul(
                                psv,
                                xt[:, dc, ts(tb, P)],
                                wv_sb[:, dc, :],
                                start=(dc == 0),
                                stop=False,
                            )
                        nc.tensor.matmul(
                            psv, ones_row[0:1, 0:P], bv_row,
                            start=False, stop=True,
                        )
                        nc.vector.tensor_copy(
                            v[:, gtb, :, 0:c.HD],
                            psv.rearrange("p (h d) -> p h d", d=c.HD),
                        )

            # ============ attention + AllGather + proj ============
            with (
                tc.tile_pool(name="pt", bufs=6) as ptp,
                tc.tile_pool(name="yt", bufs=2) as ytp,
                tc.tile_pool(name="yu", bufs=2) as yup,
                tc.tile_pool(name="lr", bufs=2) as lrp,
                tc.tile_pool(name="yag", bufs=2) as yagp,
                tc.tile_pool(name="osb", bufs=2) as osbp,
            ):
                for qt in range(c.NQT):
                    yt_q = ytp.tile([P, c.NHP, c.QT], BF16)
                    yu_q = yup.tile([P, c.NHP, c.QT], F32)
                    l_all = lrp.tile([c.H_LOC, c.QT], F32, tag="lall")
                    nkb = (qt + 1) * c.QT // P
                    for hp in range(c.NHP):
                        # the HPG heads sharing this partition group run
                        # their S^T matmuls in disjoint PE row groups
                        # (tile_position auto-derived from base_partition),
                        # so consecutive issues execute concurrently.
                        psys = [ps_y.tile([c.HD + 1, c.QT], F32, tag="psy",
                                          name=f"psy{hs}")
                                for hs in range(c.HPG)]
                        def issue_avs(st):
                            pt_l, kb2a, blocksa = st
                            for j in range(blocksa):
                                kb = kb2a + j
                                off = max(0, kb * P - qt * c.QT)
                                for hs in range(c.HPG):
                                    nc.tensor.matmul(
                                        psys[hs][:, off:],
                                        v[:, kb, hp * c.HPG + hs, :],
                                        pt_l[hs][:, j, off:],
                                        start=(kb == 0),
                                        stop=(kb == nkb - 1),
                                    )

                        pending = None
                        for kb2 in range(0, nkb, 2):
                            blocks = min(2, nkb - kb2)
                            pss_l, pt_l = [], []
                            for hs in range(c.HPG):
                                pb = hs * c.HD
                                grp = []
                                for j in range(blocks):
                                    off = max(0, (kb2 + j) * P
                                              - qt * c.QT)
                                    pss = ps_s.tile([P, c.QT], F32,
                                                    tag="pss",
                                                    name=f"pss{hs}_{j}")
                                    nc.tensor.matmul(
                                        pss[:, off:],
                                        kT[pb:pb + c.HD, hp,
                                           ts(kb2 + j, P)],
                                        qT[pb:pb + c.HD, hp,
                                           ds(qt * c.QT + off,
                                              c.QT - off)],
                                        start=True, stop=True,
                                    )
                                    grp.append(pss)
                                pss_l.append(grp)
                            # software pipeline: the previous pair's AV
                            # matmuls issue after this pair's S^T, so the
                            # exp (ScalarE) latency stays off the PE
                            # instruction stream's critical path
                            if pending is not None:
                                issue_avs(pending)
                            for hs in range(c.HPG):
                                pt = ptp.tile([P, 2, c.QT], BF16,
                                              tag="pt", name=f"pt{hs}")
                                for j in range(blocks):
                                    off = (kb2 + j) * P - qt * c.QT
                                    if off < 0:
                                        nc.scalar.activation(
                                            pt[:, j, :], pss_l[hs][j],
                                            AF.Exp, scale=c.scale)
                                        continue
                                    # cols < off are fully masked and
                                    # never read downstream
                                    nc.scalar.activation(
                                        pt[:, j, off:],
                                        pss_l[hs][j][:, off:],
                                        AF.Exp, scale=c.scale)
                                    # triangle mask on the 128 cols at
                                    # the diagonal (offset-0 pattern)
                                    nc.vector.tensor_tensor(
                                        pt[:, j, off:off + P],
                                        pt[:, j, off:off + P],
                                        mask4[:, 0, 0:P],
                                        ALU.mult,
                                    )
                                pt_l.append(pt)
                            pending = (pt_l, kb2, blocks)
                        issue_avs(pending)
                        # stage unnormalized y^T + the denominators
                        for hs in range(c.HPG):
                            pb = hs * c.HD
                            nc.vector.tensor_copy(
                                yu_q[pb:pb + c.HD, hp, :],
                                psys[hs][0:c.HD, :])
                            l_sb = lrp.tile([1, c.QT], F32, tag="lsb",
                                            name="l_sb")
                            nc.vector.tensor_copy(
                                l_sb, psys[hs][c.HD:c.HD + 1, :])
                            nc.gpsimd.dma_start(
                                l_all[hp * c.HPG + hs:
                                      hp * c.HPG + hs + 1, :], l_sb)

                    # batched softmax normalization for all 8 heads
                    r_all = lrp.tile([c.H_LOC, c.QT], BF16, tag="rall")
                    with nc.allow_low_precision(
                        reason="1/l rounded to bf16 for the broadcast matmul"
                    ):
                        nc.vector.reciprocal(r_all, l_all)
                    for hp in range(c.NHP):
                        psr = ps_mm.tile([P, max(c.TT, c.DH)], F32,
                                         tag="mm", name="psr")[:, :c.QT]
                        nc.tensor.matmul(
                            psr, esel_sb[:, ts(hp, P)], r_all,
                            start=True, stop=True,
                        )
                        nc.vector.tensor_tensor(
                            yt_q[:, hp, :], yu_q[:, hp, :], psr, ALU.mult,
                        )

                    # ship y^T tile; pairwise AllGather along the dims axis
                    y_loc = drp.tile([c.DH, c.QT], BF16, tag="yloc")
                    nc.sync.dma_start(
                        y_loc.rearrange("(hp p) t -> p hp t", p=P), yt_q
                    )
                    if use_cc:
                        y_ag = drp.tile([c.GDH, c.QT], BF16, tag="ygat")
                        nc.gpsimd.collective_compute(
                            "AllGather", ALU.bypass,
                            replica_groups=groups,
                            ins=[y_loc.opt()], outs=[y_ag.opt()],
                        )
                    else:
                        y_ag = y_loc

                    # proj (column-sharded): out rows of this query tile
                    yag_sb = yagp.tile([P, c.GDH // P, c.QT], BF16)
                    nc.sync.dma_start(
                        yag_sb, y_ag.rearrange("(ch p) t -> p ch t", p=P)
                    )
                    for tb in range(c.QT // P):
                        pso = ps_mm.tile([P, max(c.TT, c.DH)], F32,
                                         tag="mm", name="pso")[:, :c.DH]
                        for c2 in range(c.GDH // P):
                            nc.tensor.matmul(
                                pso,
                                yag_sb[:, c2, ts(tb, P)],
                                wp_sb[:, c2, :],
                                start=(c2 == 0), stop=False,
                            )
                        nc.tensor.matmul(
                            pso, ones_row[0:1, 0:P], bp_row,
                            start=False, stop=True,
                        )
                        osb = osbp.tile([P, c.DH], F32)
                        nc.vector.tensor_copy(osb, pso)
                        nc.gpsimd.dma_start(
                            out[ds(qt * c.QT + tb * P, P), :], osb)

    nc.compile()
    return nc


def shard_inputs(c: Cfg, x, w_qkv, b_qkv, w_proj, b_proj, n_cores=8):
    """Full fp32 inputs -> per-core input maps (host-side marshalling).

    Matmul operands are cast to bf16 on the host; q/k biases stay fp32
    (applied via ScalarE's per-partition bias port on the f32 PSUM)."""
    D, DH = c.D, c.DH
    oc = max(128, (c.T // 128) * c.H_LOC)
    ones = np.ones((128, oc), BF16NP)
    esel = np.zeros((c.H_LOC, c.NHP * 128), BF16NP)
    for h in range(c.H_LOC):
        hp, sub = h // c.HPG, h % c.HPG
        esel[h, hp * 128 + sub * c.HD: hp * 128 + (sub + 1) * c.HD] = 1
    maps = []
    for core in range(n_cores):
        b, hh = core // c.n_groups, core % c.n_groups
        sl = slice(hh * DH, (hh + 1) * DH)
        maps.append({
            "xT": np.ascontiguousarray(x[b].T).astype(BF16NP),
            "wq": np.ascontiguousarray(
                w_qkv[:, 0 * D:1 * D][:, sl]).astype(BF16NP),
            "wk": np.ascontiguousarray(
                w_qkv[:, 1 * D:2 * D][:, sl]).astype(BF16NP),
            "wv": np.ascontiguousarray(
                w_qkv[:, 2 * D:3 * D][:, sl]).astype(BF16NP),
            "bq": np.ascontiguousarray(
                b_qkv[0 * D:1 * D][sl], dtype=np.float32),
            "bk": np.ascontiguousarray(
                b_qkv[1 * D:2 * D][sl], dtype=np.float32),
            "bv": np.ascontiguousarray(
                b_qkv[2 * D:3 * D][sl]).reshape(1, DH).astype(BF16NP),
            "wp": np.ascontiguousarray(w_proj[:, sl]).astype(BF16NP),
            "bp": np.ascontiguousarray(
                b_proj[sl]).reshape(1, DH).astype(BF16NP),
            "onesin": ones,
            "esel": esel,
        })
    return maps


def gather_outputs(c: Cfg, results, n_cores=8):
    B = n_cores // c.n_groups
    out = np.empty((B, c.T, c.GDH), dtype=np.float32)
    for core in range(n_cores):
        b, hh = core // c.n_groups, core % c.n_groups
        out[b][:, hh * c.DH:(hh + 1) * c.DH] = results[core]["out"]
    return out


_NC_CACHE: dict = {}


def kernel(**inputs) -> np.ndarray:
    from concourse.bass_utils import run_bass_kernel_spmd

    c = FULL
    n_cores = 8
    key = (c, n_cores)
    if key not in _NC_CACHE:
        _NC_CACHE[key] = build_nc(c, n_cores)
    nc = _NC_CACHE[key]
    in_maps = shard_inputs(
        c, inputs["x"], inputs["w_qkv"], inputs["b_qkv"],
        inputs["w_proj"], inputs["b_proj"], n_cores,
    )
    res = run_bass_kernel_spmd(
        nc, in_maps, core_ids=list(range(n_cores)),
        trace=bool(int(os.environ.get("KERNEL_TRACE", "0"))),
    )
    kernel.last_results = res
    return gather_outputs(c, res.results, n_cores)


# revision 24
# speedup vs baseline: 1.0090x; 1.0090x over previous
"""Causal self-attention (B=4, T=2048, D=1024, H=16) on 8 trn2 NeuronCores.

Sharding: batch (4-way) x head-half (2-way tensor parallel) => 8 cores,
one uniform SPMD program (per-core differences are pure data: which batch's
x, which half of the QKV columns / proj columns each core receives).

Per core (batch b, head-half hh, 8 local heads), all matmul operands bf16
(fp32 PSUM accumulation; the fp32 alternative on trn2 PE is the fp32-
emulation path at 3-4 cycles/row, and f32r is its truncated high pass --
same effective operand precision as bf16 but 3x slower):
  1. QKV: q^T/k^T computed in [qkv_col, token] layout (lhsT = W chunk,
     rhs = x^T chunk); v computed in [token, vcol] layout.
  2. Attention per head, per 512-wide query tile, streaming 128-wide key
     blocks (block-causal; fully-masked key blocks are skipped):
       S^T[k,q]  = matmul(lhsT=k^T chunk, rhs=q^T tile)      (PSUM f32)
       P^T       = exp(S^T / 8) on ScalarE (PSUM -> SBUF bf16)
       diagonal blocks: causal zeroing via gpsimd affine_select
       y_ext^T  += matmul(lhsT=v_ext block, rhs=P^T): v_ext carries a ones
                   column, so row HD of the accumulator is the softmax
                   denominator l -- no extra reduction pass.
       normalization: l rows of all 8 heads batched into one DVE
       reciprocal, broadcast across partitions via a K=1 ones matmul,
       one DVE multiply per head.
  3. Pairwise AllGather of y^T (bf16, 512x512 per query tile) between the
     two cores sharing a batch => full y^T [1024, 512] on both.
  4. proj: out[:, 512 cols of this core] = y @ W_p[:, cols] (+bias),
     column-sharded => the host only concatenates, no reduction anywhere.
"""

import os
import sys
from dataclasses import dataclass

import ml_dtypes
import numpy as np

sys.path.insert(0, "/opt/trn_rl_repo")

import concourse.mybir as mybir  # noqa: E402
import concourse.tile as tile  # noqa: E402
from concourse import bacc  # noqa: E402
from concourse.bass import ds, ts  # noqa: E402

P = 128
F32 = mybir.dt.float32
BF16 = mybir.dt.bfloat16
AF = mybir.ActivationFunctionType
ALU = mybir.AluOpType
BF16NP = ml_dtypes.bfloat16


@dataclass(frozen=True)
class Cfg:
    T: int = 2048          # sequence length
    D: int = 1024          # model dim (QKV contraction dim)
    H_LOC: int = 8         # heads per core
    HD: int = 64           # head dim
    TT: int = 512          # token tile width in the QKV phase
    QT: int = 512          # query tile width in the attention phase
    n_groups: int = 2      # cores sharing a batch (pairwise AllGather)
    scale: float = 64 ** -0.5

    @property
    def DH(self):          # local head dims (y^T rows contributed per core)
        return self.H_LOC * self.HD

    @property
    def GDH(self):         # proj contraction dim (= model dim)
        return self.n_groups * self.DH

    @property
    def DCH(self):
        return self.D // P

    @property
    def NHP(self):         # 128-partition groups of local head dims
        return self.DH // P

    @property
    def HPG(self):         # heads per 128-partition group
        return P // self.HD

    @property
    def NTT(self):
        return self.T // self.TT

    @property
    def NQT(self):
        return self.T // self.QT

    @property
    def CB(self):          # 128-wide column blocks of the local q/k cols
        return self.DH // P


FULL = Cfg()


def build_nc(c: Cfg, n_cores: int = 8, with_bias: bool = True):
    """Build the (uniform SPMD) Bass program for one core."""
    assert c.T % c.TT == 0 and c.T % c.QT == 0 and c.QT % P == 0
    assert c.D % P == 0 and c.DH % P == 0 and c.TT % P == 0
    use_cc = c.n_groups > 1

    nc = bacc.Bacc(
        "TRN2", target_bir_lowering=False, debug=False, num_devices=n_cores
    )
    xT = nc.dram_tensor("xT", [c.D, c.T], BF16, kind="ExternalInput").ap()
    wq = nc.dram_tensor("wq", [c.D, c.DH], BF16, kind="ExternalInput").ap()
    wk = nc.dram_tensor("wk", [c.D, c.DH], BF16, kind="ExternalInput").ap()
    wv = nc.dram_tensor("wv", [c.D, c.DH], BF16, kind="ExternalInput").ap()
    bq = nc.dram_tensor("bq", [c.DH], F32, kind="ExternalInput").ap()
    bk = nc.dram_tensor("bk", [c.DH], F32, kind="ExternalInput").ap()
    bv = nc.dram_tensor("bv", [1, c.DH], BF16, kind="ExternalInput").ap()
    wp = nc.dram_tensor("wp", [c.GDH, c.DH], BF16, kind="ExternalInput").ap()
    bp = nc.dram_tensor("bp", [1, c.DH], BF16, kind="ExternalInput").ap()
    oc = max(P, (c.T // P) * c.H_LOC)
    onesin = nc.dram_tensor("onesin", [P, oc], BF16, kind="ExternalInput").ap()
    esel = nc.dram_tensor("esel", [c.H_LOC, c.NHP * P], BF16,
                          kind="ExternalInput").ap()
    out = nc.dram_tensor("out", [c.T, c.DH], F32, kind="ExternalOutput").ap()

    groups = [[g * c.n_groups + i for i in range(c.n_groups)]
              for g in range(max(1, n_cores // c.n_groups))]

    with tile.TileContext(nc) as tc:
        with (
            tc.tile_pool(name="const", bufs=1) as cst,
            tc.tile_pool(name="kv", bufs=1) as kv,
            tc.tile_pool(name="wproj", bufs=1) as wpp,
            tc.tile_pool(name="ps_mm", bufs=2, space="PSUM") as ps_mm,
            tc.tile_pool(name="ps_s", bufs=4, space="PSUM") as ps_s,
            tc.tile_pool(name="ps_y", bufs=2, space="PSUM") as ps_y,
            tc.tile_pool(name="dram", bufs=2, space="DRAM") as drp,
        ):
            # ---- constants ----
            ones_row = cst.tile([1, P], BF16)
            nc.gpsimd.dma_start(ones_row, onesin[0:1, 0:P])
            bq_sb = cst.tile([P, c.CB], F32)
            nc.scalar.dma_start(bq_sb, bq.rearrange("(cb p) -> p cb", p=P))
            bk_sb = cst.tile([P, c.CB], F32)
            nc.scalar.dma_start(bk_sb, bk.rearrange("(cb p) -> p cb", p=P))
            bv_row = cst.tile([1, c.DH], BF16)
            nc.scalar.dma_start(bv_row, bv)
            bp_row = cst.tile([1, c.DH], BF16)
            nc.scalar.dma_start(bp_row, bp)
            esel_sb = cst.tile([c.H_LOC, c.NHP * P], BF16)
            nc.gpsimd.dma_start(esel_sb, esel)
            # causal masks for the diagonal-region key blocks: mask[o][k,j]
            # keeps where j - k >= o*128 (j = query within tile, k = key)
            ndiag = c.QT // P
            mask4 = cst.tile([P, ndiag, c.QT], BF16)
            nc.vector.memset(mask4, 1.0)
            for o in range(ndiag):
                nc.gpsimd.affine_select(
                    mask4[:, o, :], mask4[:, o, :],
                    compare_op=ALU.is_ge, fill=0.0, base=-o * P,
                    pattern=[[1, c.QT]], channel_multiplier=-1,
                )

            # ---- persistent K^T / Q^T / V(+ones) ----
            kT = kv.tile([P, c.NHP, c.T], BF16)
            qT = kv.tile([P, c.NHP, c.T], BF16)
            v = kv.tile([P, c.T // P, c.H_LOC, c.HD + 1], BF16)
            nc.vector.memset(v[:, :, :, c.HD:c.HD + 1], 1.0)

            wp_sb = wpp.tile([P, c.GDH // P, c.DH], BF16)
            nc.gpsimd.dma_start(
                wp_sb, wp.rearrange("(ch p) n -> p ch n", p=P))

            # ================= QKV phase =================
            with (
                tc.tile_pool(name="wqkv", bufs=1) as wqk,
                tc.tile_pool(name="xt", bufs=2) as xtp,
            ):
                wq_sb = wqk.tile([P, c.DCH, c.DH], BF16)
                wk_sb = wqk.tile([P, c.DCH, c.DH], BF16)
                wv_sb = wqk.tile([P, c.DCH, c.DH], BF16)
                wr = {"wq": wq.rearrange("(ch p) n -> p ch n", p=P),
                      "wk": wk.rearrange("(ch p) n -> p ch n", p=P),
                      "wv": wv.rearrange("(ch p) n -> p ch n", p=P)}
                for dc in range(c.DCH):
                    nc.gpsimd.dma_start(wk_sb[:, dc, :], wr["wk"][:, dc, :])
                    nc.scalar.dma_start(wq_sb[:, dc, :], wr["wq"][:, dc, :])
                    nc.gpsimd.dma_start(wv_sb[:, dc, :], wr["wv"][:, dc, :])

                xT_r = xT.rearrange("(ch p) t -> p ch t", p=P)
                for tt in range(c.NTT):
                    xt = xtp.tile([P, c.DCH, c.TT], BF16)
                    nc.sync.dma_start(xt, xT_r[:, :, ts(tt, c.TT)])

                    # K^T and Q^T: [col, token] layout
                    for dst, w_sb, b_sb in (
                        (kT, wk_sb, bk_sb),
                        (qT, wq_sb, bq_sb),
                    ):
                        for cb in range(c.CB):
                            pst = ps_mm.tile([P, max(c.TT, c.DH)], F32,
                                             tag="mm", name="pst")[:, :c.TT]
                            for dc in range(c.DCH):
                                nc.tensor.matmul(
                                    pst,
                                    w_sb[:, dc, ts(cb, P)],
                                    xt[:, dc, :],
                                    start=(dc == 0),
                                    stop=(dc == c.DCH - 1),
                                )
                            nc.vector.tensor_tensor(
                                dst[:, cb, ts(tt, c.TT)], pst,
                                b_sb[:, cb:cb + 1].to_broadcast(
                                    (P, c.TT)),
                                ALU.add,
                            )

                    # V: [token, vcol] layout (+ bias via ones-row matmul)
                    for tb in range(c.TT // P):
                        gtb = tt * (c.TT // P) + tb
                        psv = ps_mm.tile([P, max(c.TT, c.DH)], F32,
                                         tag="mm", name="psv")[:, :c.DH]
                        for dc in range(c.DCH):
                            nc.tensor.matmul(
                                psv,
                                xt[:, dc, ts(tb, P)],
                                wv_sb[:, dc, :],
                                start=(dc == 0),
                                stop=(not with_bias
                                      and dc == c.DCH - 1),
                            )
                        if with_bias:
                            nc.tensor.matmul(
                                psv, ones_row[0:1, 0:P], bv_row,
                                start=False, stop=True,
                            )
                        nc.vector.tensor_copy(
                            v[:, gtb, :, 0:c.HD],
                            psv.rearrange("p (h d) -> p h d", d=c.HD),
                        )

            # ============ attention + AllGather + proj ============
            with (
                tc.tile_pool(name="pt", bufs=6) as ptp,
                tc.tile_pool(name="yt", bufs=2) as ytp,
                tc.tile_pool(name="yu", bufs=2) as yup,
                tc.tile_pool(name="lr", bufs=2) as lrp,
                tc.tile_pool(name="yag", bufs=2) as yagp,
                tc.tile_pool(name="osb", bufs=2) as osbp,
            ):
                for qt in range(c.NQT):
                    yt_q = ytp.tile([P, c.NHP, c.QT], BF16)
                    yu_q = yup.tile([P, c.NHP, c.QT], F32)
                    l_all = lrp.tile([c.H_LOC, c.QT], F32, tag="lall")
                    nkb = (qt + 1) * c.QT // P
                    for hp in range(c.NHP):
                        # the HPG heads sharing this partition group run
                        # their S^T matmuls in disjoint PE row groups
                        # (tile_position auto-derived from base_partition),
                        # so consecutive issues execute concurrently.
                        psys = [ps_y.tile([c.HD + 1, c.QT], F32, tag="psy",
                                          name=f"psy{hs}")
                                for hs in range(c.HPG)]
                        for kb2 in range(0, nkb, 2):
                            blocks = min(2, nkb - kb2)
                            pss_l, pt_l = [], []
                            for hs in range(c.HPG):
                                pb = hs * c.HD
                                grp = []
                                for j in range(blocks):
                                    off = max(0, (kb2 + j) * P
                                              - qt * c.QT)
                                    pss = ps_s.tile([P, c.QT], F32,
                                                    tag="pss",
                                                    name=f"pss{hs}_{j}")
                                    nc.tensor.matmul(
                                        pss[:, off:],
                                        kT[pb:pb + c.HD, hp,
                                           ts(kb2 + j, P)],
                                        qT[pb:pb + c.HD, hp,
                                           ds(qt * c.QT + off,
                                              c.QT - off)],
                                        start=True, stop=True,
                                    )
                                    grp.append(pss)
                                pss_l.append(grp)
                            for hs in range(c.HPG):
                                pt = ptp.tile([P, 2, c.QT], BF16,
                                              tag="pt", name=f"pt{hs}")
                                for j in range(blocks):
                                    off = (kb2 + j) * P - qt * c.QT
                                    if off < 0:
                                        nc.scalar.activation(
                                            pt[:, j, :], pss_l[hs][j],
                                            AF.Exp, scale=c.scale)
                                        continue
                                    # cols < off are fully masked and
                                    # never read downstream
                                    nc.scalar.activation(
                                        pt[:, j, off:],
                                        pss_l[hs][j][:, off:],
                                        AF.Exp, scale=c.scale)
                                    # triangle mask on the 128 cols at
                                    # the diagonal (offset-0 pattern)
                                    nc.vector.tensor_tensor(
                                        pt[:, j, off:off + P],
                                        pt[:, j, off:off + P],
                                        mask4[:, 0, 0:P],
                                        ALU.mult,
                                    )
                                pt_l.append(pt)
                            for j in range(blocks):
                                kb = kb2 + j
                                off = max(0, kb * P - qt * c.QT)
                                for hs in range(c.HPG):
                                    nc.tensor.matmul(
                                        psys[hs][:, off:],
                                        v[:, kb, hp * c.HPG + hs, :],
                                        pt_l[hs][:, j, off:],
                                        start=(kb == 0),
                                        stop=(kb == nkb - 1),
                                    )
                        # stage unnormalized y^T + the denominators
                        for hs in range(c.HPG):
                            pb = hs * c.HD
                            nc.vector.tensor_copy(
                                yu_q[pb:pb + c.HD, hp, :],
                                psys[hs][0:c.HD, :])
                            l_sb = lrp.tile([1, c.QT], F32, tag="lsb",
                                            name="l_sb")
                            nc.vector.tensor_copy(
                                l_sb, psys[hs][c.HD:c.HD + 1, :])
                            nc.gpsimd.dma_start(
                                l_all[hp * c.HPG + hs:
                                      hp * c.HPG + hs + 1, :], l_sb)

                    # batched softmax normalization for all 8 heads
                    r_all = lrp.tile([c.H_LOC, c.QT], BF16, tag="rall")
                    with nc.allow_low_precision(
                        reason="1/l rounded to bf16 for the broadcast matmul"
                    ):
                        nc.vector.reciprocal(r_all, l_all)
                    for hp in range(c.NHP):
                        psr = ps_mm.tile([P, max(c.TT, c.DH)], F32,
                                         tag="mm", name="psr")[:, :c.QT]
                        nc.tensor.matmul(
                            psr, esel_sb[:, ts(hp, P)], r_all,
                            start=True, stop=True,
                        )
                        nc.vector.tensor_tensor(
                            yt_q[:, hp, :], yu_q[:, hp, :], psr, ALU.mult,
                        )

                    # ship y^T; pairwise AllGather along the dims axis,
                    # then the column-sharded proj. The last query tile is
                    # split in two token halves so its proj overlaps the
                    # second half's AllGather (shrinks the kernel tail).
                    halves = 2 if qt == c.NQT - 1 else 1
                    hw_ = c.QT // halves
                    for hf in range(halves):
                        tsl = ds(hf * hw_, hw_)
                        y_loc = drp.tile([c.DH, hw_], BF16,
                                         tag=f"yloc{halves}", name="y_loc")
                        nc.sync.dma_start(
                            y_loc.rearrange("(hp p) t -> p hp t", p=P),
                            yt_q[:, :, tsl],
                        )
                        if use_cc:
                            y_ag = drp.tile([c.GDH, hw_], BF16,
                                            tag=f"ygat{halves}",
                                            name="y_ag")
                            nc.gpsimd.collective_compute(
                                "AllGather", ALU.bypass,
                                replica_groups=groups,
                                ins=[y_loc.opt()], outs=[y_ag.opt()],
                            )
                        else:
                            y_ag = y_loc
                        yag_sb = yagp.tile([P, c.GDH // P, c.QT], BF16,
                                           name="yag_sb")[:, :, :hw_]
                        nc.sync.dma_start(
                            yag_sb,
                            y_ag.rearrange("(ch p) t -> p ch t", p=P),
                        )
                        for tb in range(hw_ // P):
                            gtb = hf * (hw_ // P) + tb
                            pso = ps_mm.tile([P, max(c.TT, c.DH)], F32,
                                             tag="mm", name="pso")[:, :c.DH]
                            for c2 in range(c.GDH // P):
                                nc.tensor.matmul(
                                    pso,
                                    yag_sb[:, c2, ts(tb, P)],
                                    wp_sb[:, c2, :],
                                    start=(c2 == 0),
                                    stop=(not with_bias
                                          and c2 == c.GDH // P - 1),
                                )
                            if with_bias:
                                nc.tensor.matmul(
                                    pso, ones_row[0:1, 0:P], bp_row,
                                    start=False, stop=True,
                                )
                            osb = osbp.tile([P, c.DH], F32)
                            nc.vector.tensor_copy(osb, pso)
                            nc.gpsimd.dma_start(
                                out[ds(qt * c.QT + gtb * P, P), :], osb)

    nc.compile()
    return nc


def shard_inputs(c: Cfg, x, w_qkv, b_qkv, w_proj, b_proj, n_cores=8):
    """Full fp32 inputs -> per-core input maps (host-side marshalling).

    Matmul operands are cast to bf16 on the host; q/k biases stay fp32
    (applied via ScalarE's per-partition bias port on the f32 PSUM)."""
    D, DH = c.D, c.DH
    oc = max(128, (c.T // 128) * c.H_LOC)
    ones = np.ones((128, oc), BF16NP)
    esel = np.zeros((c.H_LOC, c.NHP * 128), BF16NP)
    for h in range(c.H_LOC):
        hp, sub = h // c.HPG, h % c.HPG
        esel[h, hp * 128 + sub * c.HD: hp * 128 + (sub + 1) * c.HD] = 1
    maps = []
    for core in range(n_cores):
        b, hh = core // c.n_groups, core % c.n_groups
        sl = slice(hh * DH, (hh + 1) * DH)
        maps.append({
            "xT": np.ascontiguousarray(x[b].T).astype(BF16NP),
            "wq": np.ascontiguousarray(
                w_qkv[:, 0 * D:1 * D][:, sl]).astype(BF16NP),
            "wk": np.ascontiguousarray(
                w_qkv[:, 1 * D:2 * D][:, sl]).astype(BF16NP),
            "wv": np.ascontiguousarray(
                w_qkv[:, 2 * D:3 * D][:, sl]).astype(BF16NP),
            "bq": np.ascontiguousarray(
                b_qkv[0 * D:1 * D][sl], dtype=np.float32),
            "bk": np.ascontiguousarray(
                b_qkv[1 * D:2 * D][sl], dtype=np.float32),
            "bv": np.ascontiguousarray(
                b_qkv[2 * D:3 * D][sl]).reshape(1, DH).astype(BF16NP),
            "wp": np.ascontiguousarray(w_proj[:, sl]).astype(BF16NP),
            "bp": np.ascontiguousarray(
                b_proj[sl]).reshape(1, DH).astype(BF16NP),
            "onesin": ones,
            "esel": esel,
        })
    return maps


def gather_outputs(c: Cfg, results, n_cores=8):
    B = n_cores // c.n_groups
    out = np.empty((B, c.T, c.GDH), dtype=np.float32)
    for core in range(n_cores):
        b, hh = core // c.n_groups, core % c.n_groups
        out[b][:, hh * c.DH:(hh + 1) * c.DH] = results[core]["out"]
    return out


_NC_CACHE: dict = {}


def kernel(**inputs) -> np.ndarray:
    from concourse.bass_utils import run_bass_kernel_spmd

    c = FULL
    n_cores = 8
    wb = bool(np.any(inputs["b_qkv"]) or np.any(inputs["b_proj"]))
    key = (c, n_cores, wb)
    if key not in _NC_CACHE:
        _NC_CACHE[key] = build_nc(c, n_cores, with_bias=wb)
    nc = _NC_CACHE[key]
    in_maps = shard_inputs(
        c, inputs["x"], inputs["w_qkv"], inputs["b_qkv"],
        inputs["w_proj"], inputs["b_proj"], n_cores,
    )
    res = run_bass_kernel_spmd(
        nc, in_maps, core_ids=list(range(n_cores)),
        trace=bool(int(os.environ.get("KERNEL_TRACE", "0"))),
    )
    kernel.last_results = res
    return gather_outputs(c, res.results, n_cores)
